# revision 13
# baseline (speedup 1.0000x reference)
"""MHLA2 Trainium2 kernel — 8-core SPMD (batch x head-group sharding).

Math (per batch b, head h):
  Q=x_q@W_Q[h], K=x_k@W_K[h], V=x_v@W_V[h]          [S, 64]
  SK = softmax(K/ds) over d (row-wise)               [S, 64]
  A  = SK^T @ V                                      [64, 64]
  Bt = softmax(Q/ds) @ A                             [S, 64]
  torch-view reshape [b,h,s,d]->[b,s',f]: head h owns output rows
  s' in [h*128,(h+1)*128); Btr_h = Bt_h.reshape(128, 1024)
  out rows = Btr_h @ W_O^T                           [128, 1024]

Sharding: core c = b*2 + g handles batch b, heads g*8..g*8+7 and writes
the contiguous output block out[b, g*1024:(g+1)*1024, :].

On-chip pipeline per core (S=2048, M=1024, 8 local heads):
  ph1: K-proj (xkT resident, rotated k-accum) -> exp -> rowsum -> normalize
  ph2: V-proj per s-tile -> A accumulation (frees V tiles early)
  ph3: per f-chunk: Q-proj -> exp (unnormalized, qsum via ones column of
       A_aug) -> stage5 matmul (Bt | qsum) -> normalize -> PE transpose ->
       parity-packed BtT2 -> W_O matmuls -> direct PSUM->DRAM output DMA.
"""

import numpy as np
from contextlib import ExitStack

import concourse.bass as bass
import concourse.bacc as bacc_mod
import concourse.mybir as mybir
import concourse.tile as tile
from concourse.bass_utils import run_bass_kernel_spmd
from concourse.masks import make_identity

S = 2048
M = 1024
D = 64
HL = 8            # heads per core
NK = 8            # 128-row contraction chunks of d_model
NT = 16           # 128-token tiles of S
F32 = mybir.dt.float32
F32R = mybir.dt.float32r
AX = mybir.AxisListType
AF = mybir.ActivationFunctionType
D_SCALE = float(D) ** 0.25


def _emit(ctx, tc, nc, xqT, xkT, xvT, wq, wk, wv, woT, out_ext, dbg_a=None, dbg_sk=None, dbg_qt=None):
    xpool = ctx.enter_context(tc.tile_pool(name="x", bufs=9))
    wpool = ctx.enter_context(tc.tile_pool(name="w", bufs=8))
    wopool = ctx.enter_context(tc.tile_pool(name="wo", bufs=8))
    skpool = ctx.enter_context(tc.tile_pool(name="sk", bufs=16))
    vpool = ctx.enter_context(tc.tile_pool(name="v", bufs=3))
    qpool = ctx.enter_context(tc.tile_pool(name="qT", bufs=2))
    btpool = ctx.enter_context(tc.tile_pool(name="bt", bufs=2))
    spool = ctx.enter_context(tc.tile_pool(name="small", bufs=36))
    bnpool = ctx.enter_context(tc.tile_pool(name="bn", bufs=4))
    opool = ctx.enter_context(tc.tile_pool(name="osb", bufs=2))
    cpool = ctx.enter_context(tc.tile_pool(name="const", bufs=2))
    ppool = ctx.enter_context(tc.tile_pool(name="pbig", bufs=3, space="PSUM"))
    papool = ctx.enter_context(tc.tile_pool(name="pa", bufs=1, space="PSUM"))
    p5pool = ctx.enter_context(tc.tile_pool(name="p5", bufs=2, space="PSUM"))
    ptpool = ctx.enter_context(tc.tile_pool(name="pt", bufs=2, space="PSUM"))

    ident = cpool.tile([128, 128], F32)
    make_identity(nc, ident[:])

    def load_chunks(dram, pool, width, tag):
        tiles = []
        for k in range(NK):
            t = pool.tile([128, width], F32R, tag=tag)
            nc.gpsimd.dma_start(out=t[:], in_=dram[k * 128:(k + 1) * 128, :])
            tiles.append(t)
        return tiles

    # ---------------- phase 1: K projection + softmax ----------------
    xk_sb = load_chunks(xkT, xpool, S, "x")
    wk_sb = load_chunks(wk, wpool, 512, "w")

    sk_sb = []
    for t in range(NT):
        ps = ppool.tile([128, 512], F32, tag="pbig")
        for j in range(NK):
            k = (t + j) % NK
            nc.tensor.matmul(
                ps[:],
                xk_sb[k][:, t * 128:(t + 1) * 128],
                wk_sb[k][:],
                start=(j == 0),
                stop=(j == NK - 1),
            )
        sk = skpool.tile([128, 512], F32, tag="sk")
        nc.scalar.activation(sk[:], ps[:], AF.Exp)
        ksum = spool.tile([128, 8], F32, tag="ksum")
        nc.vector.reduce_sum(
            ksum[:], sk[:].rearrange("p (h d) -> p h d", d=D), axis=AX.X
        )
        krec = spool.tile([128, 8], F32, tag="krec")
        nc.vector.reciprocal(krec[:], ksum[:])
        for h in range(HL):
            nc.vector.tensor_scalar_mul(
                sk[:, h * D:(h + 1) * D], sk[:, h * D:(h + 1) * D],
                krec[:, h:h + 1],
            )
        sk_sb.append(sk)

    # ---------------- phase 2: V projection + A accumulation ----------------
    xv_sb = load_chunks(xvT, xpool, S, "x")
    wv_sb = load_chunks(wv, wpool, 512, "w")
    wo_sb = load_chunks(woT, wopool, M, "wo")

    pa = papool.tile([64, 512], F32, tag="pa")
    for t in range(NT):
        ps = ppool.tile([128, 512], F32, tag="pbig")
        for j in range(NK):
            k = (t + j) % NK
            nc.tensor.matmul(
                ps[:],
                xv_sb[k][:, t * 128:(t + 1) * 128],
                wv_sb[k][:],
                start=(j == 0),
                stop=(j == NK - 1),
            )
        vt = vpool.tile([128, 512], F32, tag="v")
        nc.scalar.copy(vt[:], ps[:])
        for h in range(HL):
            # One accumulation group for the whole bank: start clears the
            # entire PSUM bank, so only the very first matmul may set it.
            nc.tensor.matmul(
                pa[:, h * D:(h + 1) * D],
                sk_sb[t][:, h * D:(h + 1) * D],
                vt[:, h * D:(h + 1) * D],
                start=(t == 0 and h == 0),
                stop=(t == NT - 1 and h == HL - 1),
                skip_group_check=True,
            )

    # A_aug: per head [64, 65] = [A_h | ones]; stride-65 packing.
    # Rows 64-127 hold a copy so stage5 rhs base_partition can match the
    # lhsT slice (qt rows 64-127 for odd local heads).
    a_aug = cpool.tile([128, HL * 65], F32)
    nc.gpsimd.memset(
        a_aug[0:64, :].rearrange("p (h c) -> p h c", c=65)[:, :, 64:65], 1.0
    )
    nc.vector.tensor_copy(
        a_aug[0:64, :].rearrange("p (h c) -> p h c", c=65)[:, :, 0:64],
        pa[:].rearrange("p (h d) -> p h d", d=D),
    )
    nc.sync.dma_start(out=a_aug[64:128, :], in_=a_aug[0:64, :])
    if dbg_a is not None:
        nc.sync.dma_start(out=dbg_a[:], in_=a_aug[:])
        nc.sync.dma_start(out=dbg_sk[:], in_=sk_sb[0][:])

    # ---------------- phase 3: Q -> expQ^T -> Bt -> W_O ----------------
    xq_sb = load_chunks(xqT, xpool, S, "x")
    wq_sb = load_chunks(wq, wpool, 512, "w")

    for fc in range(4):
        qt = qpool.tile([128, S], F32, tag="qT")
        for sc in range(4):
            ps = ppool.tile([128, 512], F32, tag="pbig")
            for j in range(NK):
                k = (sc + j) % NK
                nc.tensor.matmul(
                    ps[:],
                    wq_sb[k][:, fc * 128:(fc + 1) * 128],
                    xq_sb[k][:, sc * 512:(sc + 1) * 512],
                    start=(j == 0),
                    stop=(j == NK - 1),
                )
            nc.scalar.activation(qt[:, sc * 512:(sc + 1) * 512], ps[:], AF.Exp)

        if fc == 0 and dbg_qt is not None:
            nc.sync.dma_start(out=dbg_qt[:], in_=qt[:])
        for hh in range(2):
            h = 2 * fc + hh       # local head
            bt2 = btpool.tile([128, M], F32R, tag="bt")
            for t in range(NT):
                p5 = p5pool.tile([128, 65], F32, tag="p5")
                nc.tensor.matmul(
                    p5[:],
                    qt[hh * 64:(hh + 1) * 64, t * 128:(t + 1) * 128],
                    a_aug[hh * 64:(hh + 1) * 64, h * 65:(h + 1) * 65],
                    start=True,
                    stop=True,
                )
                qrec = spool.tile([128, 1], F32, tag="qrec")
                nc.vector.reciprocal(qrec[:], p5[:, 64:65])
                bn = bnpool.tile([128, 64], F32, tag="bn")
                nc.vector.tensor_scalar_mul(bn[:], p5[:, 0:64], qrec[:])
                pt = ptpool.tile([64, 128], F32, tag="pt")
                nc.tensor.transpose(
                    pt[:], bn[:],
                    ident[:],
                )
                ptv = pt[:].rearrange("p (q two) -> p two q", two=2)
                eng = nc.scalar if (t % 2 == 0) else nc.vector
                if t % 2 == 0:
                    nc.scalar.copy(bt2[0:64, t * 64:(t + 1) * 64], ptv[:, 0, :])
                    nc.vector.tensor_copy(
                        bt2[64:128, t * 64:(t + 1) * 64], ptv[:, 1, :]
                    )
                else:
                    nc.vector.tensor_copy(
                        bt2[0:64, t * 64:(t + 1) * 64], ptv[:, 0, :]
                    )
                    nc.scalar.copy(bt2[64:128, t * 64:(t + 1) * 64], ptv[:, 1, :])

            bt2v = bt2[:].rearrange("p (q c) -> p c q", c=8)
            for oh in range(2):
                po = ppool.tile([128, 512], F32, tag="pbig")
                for c in range(NK):
                    nc.tensor.matmul(
                        po[:],
                        bt2v[:, c, :],
                        wo_sb[c][:, oh * 512:(oh + 1) * 512],
                        start=(c == 0),
                        stop=(c == NK - 1),
                    )
                ob = opool.tile([128, 512], F32, tag="osb")
                nc.scalar.copy(ob[:], po[:])
                nc.sync.dma_start(
                    out=out_ext[h * 128:(h + 1) * 128, oh * 512:(oh + 1) * 512],
                    in_=ob[:],
                )


_NC_CACHE = None


def _build():
    global _NC_CACHE
    if _NC_CACHE is not None:
        return _NC_CACHE
    nc = bacc_mod.Bacc(None, target_bir_lowering=False)
    xqT = nc.declare_dram_parameter("xqT", [M, S], F32R, isOutput=False)
    xkT = nc.declare_dram_parameter("xkT", [M, S], F32R, isOutput=False)
    xvT = nc.declare_dram_parameter("xvT", [M, S], F32R, isOutput=False)
    wq = nc.declare_dram_parameter("wq", [M, 512], F32R, isOutput=False)
    wk = nc.declare_dram_parameter("wk", [M, 512], F32R, isOutput=False)
    wv = nc.declare_dram_parameter("wv", [M, 512], F32R, isOutput=False)
    woT = nc.declare_dram_parameter("woT", [M, M], F32R, isOutput=False)
    out = nc.declare_dram_parameter("out", [HL * 128, M], F32, isOutput=True)
    dbg_a = nc.declare_dram_parameter("dbg_a", [128, HL * 65], F32, isOutput=True)
    dbg_sk = nc.declare_dram_parameter("dbg_sk", [128, 512], F32, isOutput=True)
    dbg_qt = nc.declare_dram_parameter("dbg_qt", [128, S], F32, isOutput=True)
    with tile.TileContext(nc) as tc, ExitStack() as ctx:
        _emit(ctx, tc, nc, xqT, xkT, xvT, wq, wk, wv, woT, out, dbg_a, dbg_sk, dbg_qt)
    if not nc.is_finalized():
        nc.finalize()
    _NC_CACHE = nc
    return nc


def _in_maps(x_q, x_k, x_v, W_Q, W_K, W_V, W_O):
    woT = np.ascontiguousarray(W_O.T.astype(np.float32))
    maps = []
    for b in range(4):
        xqT = np.ascontiguousarray(x_q[b].T)
        xkT = np.ascontiguousarray(x_k[b].T)
        xvT = np.ascontiguousarray(x_v[b].T)
        for g in range(2):
            sl = slice(g * HL, (g + 1) * HL)
            maps.append({
                "xqT": xqT, "xkT": xkT, "xvT": xvT,
                "wq": np.ascontiguousarray(
                    (W_Q[sl] / D_SCALE).transpose(1, 0, 2).reshape(M, 512)),
                "wk": np.ascontiguousarray(
                    (W_K[sl] / D_SCALE).transpose(1, 0, 2).reshape(M, 512)),
                "wv": np.ascontiguousarray(
                    W_V[sl].transpose(1, 0, 2).reshape(M, 512)),
                "woT": woT,
            })
    return maps


def run(inputs, **kw):
    nc = _build()
    maps = _in_maps(inputs["x_q"], inputs["x_k"], inputs["x_v"],
                    inputs["W_Q"], inputs["W_K"], inputs["W_V"],
                    inputs["W_O"])
    res = run_bass_kernel_spmd(nc, maps, list(range(8)), **kw)
    out = np.empty((4, S, M), dtype=np.float32)
    for b in range(4):
        for g in range(2):
            out[b, g * M:(g + 1) * M, :] = res.results[b * 2 + g]["out"]
    return out, res


def kernel(**inputs):
    out, _ = run(inputs)
    return out
